# revision 4
# baseline (speedup 1.0000x reference)
"""Trainium2 Bass kernel for nn_CASAtt_MultiHead_v1 (CAS attention block).

Reference computation (per sample):
    qkv = 1x1 conv (qkv_w) -> q, k, v                        [512, 56, 56] each
    q <- SE(dwconv3x3(q, sq_w, sq_b))   (per-head squeeze-excite)
    k <- SE(dwconv3x3(k, sk_w, sk_b))
    out = proj(dwconv3x3(q + k, dwc_w, dwc_b) * v) + proj_b + x

Distribution: data-parallel over batch, 2 samples per NeuronCore x 8 cores.

Layout: channels on partitions, 4 chunks of 128 (chunk == SE head).
Depthwise convs: 9 taps, each tap applied either as a diagonal-matrix matmul
accumulated in PSUM (TensorE) or as a fused scalar MAC chain
(scalar_tensor_tensor) on VectorE/GpSimd.  SE average-pool is fused into the
conv1 PSUM drain via accum_out.  Since depthwise conv and the SE scale are
per-channel linear, m = s_q*dwq + s_k*dwk is built after both branches and a
single third conv runs on m.
"""

import numpy as np

DIM = 512
NH = 4
HD = 128
HD4 = 32
B, H_FULL, W = 16, 56, 56
N_CORES = 8

TAPS = [(dy, dx) for dy in (-1, 0, 1) for dx in (-1, 0, 1)]


def default_cfg():
    return dict(
        b_local=B // N_CORES,
        H=H_FULL,
        rows_per_tile=8,
        # dtype of the padded conv-domain buffers (qpad / dwq / dwk)
        conv_bf16=True,
        # engine per (branch, oc) for conv1:  'pe' | 'dve' | 'gps'
        conv1_assign={(br, oc): 'pe' for br in (0, 1) for oc in range(NH)},
        # engine per oc for conv2 (on m): only 'pe' supported
        conv2_assign={oc: 'pe' for oc in range(NH)},
        repeat=1,
    )


def _np(dt):
    import concourse.mybir as mybir
    return mybir.dt.np(dt)


def build_nc(cfg):
    """Build + compile the Bacc program for one core (SPMD across 8)."""
    import concourse.bass as bass
    import concourse.mybir as mybir
    import concourse.tile as tile
    from concourse import bacc
    from contextlib import ExitStack

    f32 = mybir.dt.float32
    bf16 = mybir.dt.bfloat16
    cdt = bf16 if cfg['conv_bf16'] else f32

    BL = cfg['b_local']
    H = cfg['H']
    TH = cfg['rows_per_tile']
    NT = H // TH
    assert NT * TH == H
    TN = TH * W                      # spatial tile size (free dim of matmuls)
    HP, WP = H + 2, W + 2
    PADN = HP * WP
    NPIX = H * W
    AF = mybir.ActivationFunctionType
    AL = mybir.AluOpType

    nc = bacc.Bacc("TRN2", target_bir_lowering=False, debug=False,
                   enable_asserts=False, num_devices=N_CORES)

    # ---------------- DRAM I/O ----------------
    x_d = nc.dram_tensor("x", [BL, DIM, H, W], f32, kind="ExternalInput").ap()
    out_d = nc.dram_tensor("out", [BL, DIM, H, W], f32, kind="ExternalOutput").ap()
    wq_d = nc.dram_tensor("wq_t", [DIM, DIM], f32, kind="ExternalInput").ap()
    wk_d = nc.dram_tensor("wk_t", [DIM, DIM], f32, kind="ExternalInput").ap()
    wv_d = nc.dram_tensor("wv_t", [DIM, DIM], f32, kind="ExternalInput").ap()
    wp_d = nc.dram_tensor("proj_t", [DIM, DIM], f32, kind="ExternalInput").ap()
    # diagonal tap matrices for PE convs
    dg1_d = [nc.dram_tensor(n, [NH, 9, HD, HD], cdt, kind="ExternalInput").ap()
             for n in ("diag1q", "diag1k")]
    dg2_d = nc.dram_tensor("diag2", [NH, 9, HD, HD], cdt, kind="ExternalInput").ap()
    # tap weight vectors for DVE/GPS convs: [NH, HD, 9]
    wv1_d = [nc.dram_tensor(n, [NH, HD, 9], f32, kind="ExternalInput").ap()
             for n in ("wvec1q", "wvec1k")]
    # biases as [DIM, 1]
    b1_d = [nc.dram_tensor(n, [DIM, 1], f32, kind="ExternalInput").ap()
            for n in ("sq_b", "sk_b")]
    dwcb_d = nc.dram_tensor("dwc_b", [DIM, 1], f32, kind="ExternalInput").ap()
    projb_d = nc.dram_tensor("proj_b", [DIM, 1], f32, kind="ExternalInput").ap()
    # SE weights: w1 [NH, HD, HD4] (lhsT, prescaled by 1/npix), b1 [NH, HD4, 1]
    #             w2 [NH, HD4, HD] (lhsT), b2 [NH, HD, 1]
    sew1_d = [nc.dram_tensor(n, [NH, HD, HD4], f32, kind="ExternalInput").ap()
              for n in ("se_w1q", "se_w1k")]
    seb1_d = [nc.dram_tensor(n, [NH, HD4, 1], f32, kind="ExternalInput").ap()
              for n in ("se_b1q", "se_b1k")]
    sew2_d = [nc.dram_tensor(n, [NH, HD4, HD], f32, kind="ExternalInput").ap()
              for n in ("se_w2q", "se_w2k")]
    seb2_d = [nc.dram_tensor(n, [NH, HD, 1], f32, kind="ExternalInput").ap()
              for n in ("se_b2q", "se_b2k")]

    with tile.TileContext(nc) as tc, ExitStack() as ctx:
        const = ctx.enter_context(tc.tile_pool(name="const", bufs=1))
        big = ctx.enter_context(tc.tile_pool(name="big", bufs=1))
        wpool = ctx.enter_context(tc.tile_pool(name="wpool", bufs=1))
        xpool = ctx.enter_context(tc.tile_pool(name="xpool", bufs=2))
        vpool = ctx.enter_context(tc.tile_pool(name="vpool", bufs=2))
        o2pool = ctx.enter_context(tc.tile_pool(name="o2pool", bufs=2))
        otpool = ctx.enter_context(tc.tile_pool(name="otpool", bufs=2))
        dgpool = ctx.enter_context(tc.tile_pool(name="dgpool", bufs=1))
        accpool = ctx.enter_context(tc.tile_pool(name="accpool", bufs=1))
        statpool = ctx.enter_context(tc.tile_pool(name="statpool", bufs=2))
        mmpool = ctx.enter_context(tc.tile_pool(name="mmpool", bufs=5, space="PSUM"))
        sepool = ctx.enter_context(tc.tile_pool(name="sepool", bufs=2, space="PSUM"))

        # ---------- persistent SBUF ----------
        qpad = [big.tile([HD, PADN], cdt, name=f"qpad{c}") for c in range(NH)]
        dwq = [big.tile([HD, PADN], cdt, name=f"dwq{c}") for c in range(NH)]
        dwk = [big.tile([HD, PADN], cdt, name=f"dwk{c}") for c in range(NH)]
        qpad3 = [t.rearrange("p (h w) -> p h w", w=WP) for t in qpad]
        dwq3 = [t.rearrange("p (h w) -> p h w", w=WP) for t in dwq]
        dwk3 = [t.rearrange("p (h w) -> p h w", w=WP) for t in dwk]

        # zero the halo borders once (interiors are fully rewritten per use)
        for t3 in qpad3 + dwq3 + dwk3:
            nc.vector.memset(t3[:, 0:1, :], 0.0)          # top pad row
            nc.vector.memset(t3[:, HP - 1:HP, :], 0.0)    # bottom pad row
            nc.vector.memset(t3[:, :, 0:1], 0.0)          # left pad col
            nc.vector.memset(t3[:, :, WP - 1:WP], 0.0)    # right pad col

        # small constants
        bias1 = [[const.tile([HD, 1], f32, name=f"b1_{br}_{c}") for c in range(NH)]
                 for br in range(2)]
        dwcb = [const.tile([HD, 1], f32, name=f"dwcb{c}") for c in range(NH)]
        projb = [const.tile([HD, 1], f32, name=f"projb{c}") for c in range(NH)]
        for c in range(NH):
            sl = slice(c * HD, (c + 1) * HD)
            for br in range(2):
                nc.sync.dma_start(bias1[br][c], b1_d[br][sl])
            nc.sync.dma_start(dwcb[c], dwcb_d[sl])
            nc.sync.dma_start(projb[c], projb_d[sl])
        sew1 = [[const.tile([HD, HD4], f32, name=f"sew1_{br}_{c}") for c in range(NH)]
                for br in range(2)]
        seb1 = [[const.tile([HD4, 1], f32, name=f"seb1_{br}_{c}") for c in range(NH)]
                for br in range(2)]
        sew2 = [[const.tile([HD4, HD], f32, name=f"sew2_{br}_{c}") for c in range(NH)]
                for br in range(2)]
        seb2 = [[const.tile([HD, 1], f32, name=f"seb2_{br}_{c}") for c in range(NH)]
                for br in range(2)]
        for br in range(2):
            for c in range(NH):
                nc.sync.dma_start(sew1[br][c], sew1_d[br][c])
                nc.sync.dma_start(seb1[br][c], seb1_d[br][c])
                nc.sync.dma_start(sew2[br][c], sew2_d[br][c])
                nc.sync.dma_start(seb2[br][c], seb2_d[br][c])

        def taps_views(p3, r0):
            """9 shifted input views for output rows r0..r0+TH-1 (padded buf)."""
            return [p3[:, r0 + dy + 1: r0 + dy + 1 + TH, dx + 1: dx + 1 + W]
                    for (dy, dx) in TAPS]

        def taps_views_full(p3):
            return [p3[:, dy + 1: dy + 1 + H, dx + 1: dx + 1 + W]
                    for (dy, dx) in TAPS]

        def emit_body(rep):
            sfx = f"_r{rep}" if cfg['repeat'] > 1 else ""
            s_scale = [[None] * NH for _ in range(2)]   # sigmoid outputs

            for b in range(BL):
                # ================= PHASE 1: q then k =================
                for br in range(2):
                    w_d = wq_d if br == 0 else wk_d
                    pad3 = qpad3
                    dst3 = dwq3 if br == 0 else dwk3
                    w_sb = []
                    for kc in range(NH):
                        wt = wpool.tile([HD, DIM], f32, tag=f"wA{kc}",
                                        name=f"wA{kc}_b{b}_{br}{sfx}")
                        nc.sync.dma_start(wt, w_d[kc * HD:(kc + 1) * HD, :])
                        w_sb.append(wt)
                    # ---- GEMM: branch channels over all spatial tiles ----
                    for t in range(NT):
                        r0 = t * TH
                        xt = []
                        for kc in range(NH):
                            xx = xpool.tile([HD, TN], f32, tag=f"xt{kc}",
                                            name=f"xt{kc}_b{b}_{br}_{t}{sfx}")
                            nc.sync.dma_start(
                                xx.rearrange("p (h w) -> p h w", w=W),
                                x_d[b, kc * HD:(kc + 1) * HD, r0:r0 + TH, :])
                            xt.append(xx)
                        for oc in range(NH):
                            ps = mmpool.tile([HD, TN], f32, tag="mm",
                                             name=f"g{b}_{br}_{t}_{oc}{sfx}")
                            for kc in range(NH):
                                nc.tensor.matmul(
                                    ps, w_sb[kc][:, oc * HD:(oc + 1) * HD], xt[kc],
                                    start=(kc == 0), stop=(kc == NH - 1))
                            nc.scalar.copy(
                                pad3[oc][:, 1 + r0:1 + r0 + TH, 1:1 + W],
                                ps.rearrange("p (h w) -> p h w", w=W))
                    # ---- conv1 + fused pooling ----
                    for oc in range(NH):
                        eng = cfg['conv1_assign'][(br, oc)]
                        if eng == 'pe':
                            stats = statpool.tile([HD, NT], f32, tag="stats",
                                                  name=f"st{b}_{br}_{oc}{sfx}")
                            dgs = []
                            for j in range(9):
                                dg = dgpool.tile([HD, HD], cdt, tag=f"dg{j}",
                                                 name=f"dg{j}_{b}_{br}_{oc}{sfx}")
                                nc.sync.dma_start(dg, dg1_d[br][oc, j])
                                dgs.append(dg)
                            for t in range(NT):
                                r0 = t * TH
                                ps = mmpool.tile([HD, TN], f32, tag="mm",
                                                 name=f"c1{b}_{br}_{t}_{oc}{sfx}")
                                for j, v in enumerate(taps_views(pad3[oc], r0)):
                                    nc.tensor.matmul(ps, dgs[j], v,
                                                     start=(j == 0), stop=(j == 8))
                                nc.scalar.activation(
                                    dst3[oc][:, 1 + r0:1 + r0 + TH, 1:1 + W],
                                    ps.rearrange("p (h w) -> p h w", w=W),
                                    AF.Identity, bias=bias1[br][oc],
                                    accum_out=stats[:, t:t + 1])
                            pooled_w = NT
                        else:
                            stats = statpool.tile([HD, NT], f32, tag="stats",
                                                  name=f"st{b}_{br}_{oc}{sfx}")
                            wvec = dgpool.tile([HD, 9], f32, tag="wvec",
                                               name=f"wv{b}_{br}_{oc}{sfx}")
                            nc.sync.dma_start(wvec, wv1_d[br][oc])
                            e = nc.vector if eng == 'dve' else nc.gpsimd
                            views = taps_views_full(pad3[oc])
                            acc = [accpool.tile([HD, NPIX], cdt, tag=f"acc{i}",
                                                name=f"acc{i}_{b}_{br}_{oc}{sfx}")
                                   for i in range(2)]
                            a3 = [a.rearrange("p (h w) -> p h w", w=W) for a in acc]
                            # tap 0 (+ bias), taps 1..7 ping-pong, tap 8 -> dst
                            e.tensor_scalar(a3[0], views[0], wvec[:, 0:1],
                                            bias1[br][oc], AL.mult, AL.add)
                            cur = 0
                            for j in range(1, 8):
                                e.scalar_tensor_tensor(
                                    a3[1 - cur], views[j], wvec[:, j:j + 1],
                                    a3[cur], AL.mult, AL.add)
                                cur = 1 - cur
                            e.scalar_tensor_tensor(
                                dst3[oc][:, 1:1 + H, 1:1 + W], views[8],
                                wvec[:, 8:9], a3[cur], AL.mult, AL.add,
                                accum_out=stats[:, 0:1])
                            pooled_w = 1
                        # ---- SE for this head ----
                        pooled = const.tile([HD, 1], f32, tag="pooled",
                                            bufs=4, name=f"pool{b}_{br}_{oc}{sfx}")
                        nc.vector.tensor_reduce(pooled, stats[:, 0:pooled_w],
                                                mybir.AxisListType.X, AL.add)
                        ps1 = sepool.tile([HD4, 1], f32, tag="se",
                                          name=f"se1_{b}_{br}_{oc}{sfx}")
                        nc.tensor.matmul(ps1, sew1[br][oc], pooled,
                                         start=True, stop=True)
                        hvec = const.tile([HD4, 1], f32, tag="hvec", bufs=4,
                                          name=f"h{b}_{br}_{oc}{sfx}")
                        nc.scalar.activation(hvec, ps1, AF.Relu,
                                             bias=seb1[br][oc])
                        ps2 = sepool.tile([HD, 1], f32, tag="se",
                                          name=f"se2_{b}_{br}_{oc}{sfx}")
                        nc.tensor.matmul(ps2, sew2[br][oc], hvec,
                                         start=True, stop=True)
                        s_sb = const.tile([HD, 1], f32, tag="s_scale", bufs=16,
                                          name=f"s{b}_{br}_{oc}{sfx}")
                        nc.scalar.activation(s_sb, ps2, AF.Sigmoid,
                                             bias=seb2[br][oc])
                        s_scale[br][oc] = s_sb

                # ============ PHASE 1.5: m = s_q*dwq + s_k*dwk (into dwk) ====
                for oc in range(NH):
                    scr = accpool.tile([HD, PADN], cdt, tag="acc0",
                                       name=f"scr{b}_{oc}{sfx}")
                    nc.vector.tensor_scalar(scr, dwk[oc], s_scale[1][oc], None,
                                            AL.mult)
                    nc.vector.scalar_tensor_tensor(dwk[oc], dwq[oc],
                                                   s_scale[0][oc], scr,
                                                   AL.mult, AL.add)

                # ================= PHASE 2 =================
                wv_sb, wp_sb = [], []
                for kc in range(NH):
                    wt = wpool.tile([HD, DIM], f32, tag=f"wA{kc}",
                                    name=f"wV{kc}_b{b}{sfx}")
                    nc.sync.dma_start(wt, wv_d[kc * HD:(kc + 1) * HD, :])
                    wv_sb.append(wt)
                    wt2 = wpool.tile([HD, DIM], f32, tag=f"wB{kc}",
                                     name=f"wP{kc}_b{b}{sfx}")
                    nc.sync.dma_start(wt2, wp_d[kc * HD:(kc + 1) * HD, :])
                    wp_sb.append(wt2)
                dg2 = []
                for oc in range(NH):
                    row = []
                    for j in range(9):
                        dg = dgpool.tile([HD, HD], cdt, tag=f"dg2_{oc}_{j}",
                                         name=f"dg2_{oc}_{j}_b{b}{sfx}")
                        nc.sync.dma_start(dg, dg2_d[oc, j])
                        row.append(dg)
                    dg2.append(row)
                for t in range(NT):
                    r0 = t * TH
                    xt = []
                    for kc in range(NH):
                        xx = xpool.tile([HD, TN], f32, tag=f"xt{kc}",
                                        name=f"x2_{kc}_b{b}_{t}{sfx}")
                        nc.sync.dma_start(
                            xx.rearrange("p (h w) -> p h w", w=W),
                            x_d[b, kc * HD:(kc + 1) * HD, r0:r0 + TH, :])
                        xt.append(xx)
                    v_sb = []
                    for oc in range(NH):
                        ps = mmpool.tile([HD, TN], f32, tag="mm",
                                         name=f"v{b}_{t}_{oc}{sfx}")
                        for kc in range(NH):
                            nc.tensor.matmul(
                                ps, wv_sb[kc][:, oc * HD:(oc + 1) * HD], xt[kc],
                                start=(kc == 0), stop=(kc == NH - 1))
                        vv = vpool.tile([HD, TN], f32, tag=f"vt{oc}",
                                        name=f"vt{oc}_b{b}_{t}{sfx}")
                        nc.scalar.copy(vv, ps)
                        v_sb.append(vv)
                    o2 = []
                    for oc in range(NH):
                        ps = mmpool.tile([HD, TN], f32, tag="mm",
                                         name=f"c2{b}_{t}_{oc}{sfx}")
                        for j, v in enumerate(taps_views(dwk3[oc], r0)):
                            nc.tensor.matmul(ps, dg2[oc][j], v,
                                             start=(j == 0), stop=(j == 8))
                        oo = o2pool.tile([HD, TN], f32, tag=f"o2_{oc}",
                                         name=f"o2_{oc}_b{b}_{t}{sfx}")
                        # (conv2 + dwc_b) * v
                        nc.vector.scalar_tensor_tensor(oo, ps, dwcb[oc],
                                                       v_sb[oc], AL.add, AL.mult)
                        o2.append(oo)
                    for oc in range(NH):
                        ps = mmpool.tile([HD, TN], f32, tag="mm",
                                         name=f"p{b}_{t}_{oc}{sfx}")
                        for kc in range(NH):
                            nc.tensor.matmul(
                                ps, wp_sb[kc][:, oc * HD:(oc + 1) * HD], o2[kc],
                                start=(kc == 0), stop=(kc == NH - 1))
                        ot = otpool.tile([HD, TN], f32, tag=f"ot{oc}",
                                         name=f"ot{oc}_b{b}_{t}{sfx}")
                        # (proj + proj_b) + x  (residual)
                        nc.vector.scalar_tensor_tensor(ot, ps, projb[oc],
                                                       xt[oc], AL.add, AL.add)
                        nc.sync.dma_start(
                            out_d[b, oc * HD:(oc + 1) * HD, r0:r0 + TH, :],
                            ot.rearrange("p (h w) -> p h w", w=W))

        if cfg['repeat'] > 1:
            for rep in range(cfg['repeat']):
                emit_body(rep)
        else:
            emit_body(0)

    nc.compile()
    return nc


# ---------------------------------------------------------------------------
# host-side weight prep
# ---------------------------------------------------------------------------

def prep_weights(inputs, cfg):
    cdt = np.dtype('bfloat16') if False else None  # placeholder
    import ml_dtypes
    conv_np = ml_dtypes.bfloat16 if cfg['conv_bf16'] else np.float32
    f32 = np.float32
    qkv_w = np.asarray(inputs['qkv_w'], f32)
    wq_t = np.ascontiguousarray(qkv_w[0:DIM].T)
    wk_t = np.ascontiguousarray(qkv_w[DIM:2 * DIM].T)
    wv_t = np.ascontiguousarray(qkv_w[2 * DIM:3 * DIM].T)
    proj_t = np.ascontiguousarray(np.asarray(inputs['proj_w'], f32).T)

    def diag_taps(wconv):
        w = np.asarray(wconv, f32).reshape(DIM, 9)    # [(dy,dx) row-major]
        out = np.zeros((NH, 9, HD, HD), f32)
        idx = np.arange(HD)
        for c in range(NH):
            for j in range(9):
                out[c, j, idx, idx] = w[c * HD:(c + 1) * HD, j]
        return out.astype(conv_np)

    def wvecs(wconv):
        w = np.asarray(wconv, f32).reshape(DIM, 9)
        return np.ascontiguousarray(w.reshape(NH, HD, 9))

    npix = cfg['H'] * W
    d = dict(
        wq_t=wq_t, wk_t=wk_t, wv_t=wv_t, proj_t=proj_t,
        diag1q=diag_taps(inputs['sq_w']),
        diag1k=diag_taps(inputs['sk_w']),
        diag2=diag_taps(inputs['dwc_w']),
        wvec1q=wvecs(inputs['sq_w']),
        wvec1k=wvecs(inputs['sk_w']),
        sq_b=np.asarray(inputs['sq_b'], f32).reshape(DIM, 1),
        sk_b=np.asarray(inputs['sk_b'], f32).reshape(DIM, 1),
        dwc_b=np.asarray(inputs['dwc_b'], f32).reshape(DIM, 1),
        proj_b=np.asarray(inputs['proj_b'], f32).reshape(DIM, 1),
        se_w1q=np.ascontiguousarray(
            np.asarray(inputs['cq_w1'], f32).transpose(0, 2, 1) / npix),
        se_b1q=np.asarray(inputs['cq_b1'], f32).reshape(NH, HD4, 1),
        se_w2q=np.ascontiguousarray(
            np.asarray(inputs['cq_w2'], f32).transpose(0, 2, 1)),
        se_b2q=np.asarray(inputs['cq_b2'], f32).reshape(NH, HD, 1),
        se_w1k=np.ascontiguousarray(
            np.asarray(inputs['ck_w1'], f32).transpose(0, 2, 1) / npix),
        se_b1k=np.asarray(inputs['ck_b1'], f32).reshape(NH, HD4, 1),
        se_w2k=np.ascontiguousarray(
            np.asarray(inputs['ck_w2'], f32).transpose(0, 2, 1)),
        se_b2k=np.asarray(inputs['ck_b2'], f32).reshape(NH, HD, 1),
    )
    return d


_CACHE = {}


def _get_compiled(cfg_key, cfg):
    if cfg_key not in _CACHE:
        _CACHE[cfg_key] = build_nc(cfg)
    return _CACHE[cfg_key]


def kernel(**inputs):
    from concourse import bass_utils
    cfg = default_cfg()
    nc = _get_compiled('main', cfg)
    w = prep_weights(inputs, cfg)
    x = np.asarray(inputs['x'], np.float32)
    BL = cfg['b_local']
    in_maps = []
    for core in range(N_CORES):
        m = dict(w)
        m['x'] = np.ascontiguousarray(x[core * BL:(core + 1) * BL])
        in_maps.append(m)
    res = bass_utils.run_bass_kernel_spmd(nc, in_maps, core_ids=list(range(N_CORES)))
    out = np.empty((B, DIM, H_FULL, W), np.float32)
    for core in range(N_CORES):
        out[core * BL:(core + 1) * BL] = res.results[core]['out']
    return out


# revision 11
# speedup vs baseline: 1.6896x; 1.6896x over previous
"""Trainium2 Bass kernel for nn_CASAtt_MultiHead_v1 (CAS attention block).

Reference computation (per sample):
    qkv = 1x1 conv (qkv_w) -> q, k, v                        [512, 56, 56] each
    q <- SE(dwconv3x3(q, sq_w, sq_b))   (per-head squeeze-excite)
    k <- SE(dwconv3x3(k, sk_w, sk_b))
    out = proj(dwconv3x3(q + k, dwc_w, dwc_b) * v) + proj_b + x

Distribution: data-parallel over batch, 2 samples per NeuronCore x 8 cores.

Layout: channels on partitions, 4 chunks of 128 (chunk == SE head).
Depthwise convs: 9 taps, each tap applied either as a diagonal-matrix matmul
accumulated in PSUM (TensorE) or as a fused scalar MAC chain
(scalar_tensor_tensor) on VectorE/GpSimd.  SE average-pool is fused into the
conv1 PSUM drain via accum_out.  Since depthwise conv and the SE scale are
per-channel linear, m = s_q*dwq + s_k*dwk is built after both branches and a
single third conv runs on m.
"""

import numpy as np

DIM = 512
NH = 4
HD = 128
HD4 = 32
B, H_FULL, W = 16, 56, 56
N_CORES = 8

TAPS = [(dy, dx) for dy in (-1, 0, 1) for dx in (-1, 0, 1)]


def default_cfg():
    return dict(
        b_local=B // N_CORES,
        H=H_FULL,
        rows_per_tile=8,
        # dtype of the padded conv-domain buffers (qpad / dwq / dwk)
        conv_bf16=True,
        gemm_bf16=True,
        conv_flat=True,
        # engine per (branch, oc) for conv1:  'pe' | 'dve' | 'gps'
        conv1_assign={(br, oc): 'pe' for br in (0, 1) for oc in range(NH)},
        # engine per oc for conv2 (on m): only 'pe' supported
        conv2_assign={oc: 'pe' for oc in range(NH)},
        repeat=1,
    )


def _np(dt):
    import concourse.mybir as mybir
    return mybir.dt.np(dt)


def build_nc(cfg):
    """Build + compile the Bacc program for one core (SPMD across 8)."""
    import concourse.bass as bass
    import concourse.mybir as mybir
    import concourse.tile as tile
    from concourse import bacc
    from contextlib import ExitStack

    f32 = mybir.dt.float32
    bf16 = mybir.dt.bfloat16
    cdt = bf16 if cfg['conv_bf16'] else f32
    gdt = bf16 if cfg['gemm_bf16'] else f32

    BL = cfg['b_local']
    H = cfg['H']
    TH = cfg['rows_per_tile']
    NT = H // TH
    assert NT * TH == H
    TN = TH * W                      # spatial tile size (free dim of matmuls)
    HP, WP = H + 2, W + 2
    PADN = HP * WP
    NPIX = H * W
    AF = mybir.ActivationFunctionType
    AL = mybir.AluOpType

    nc = bacc.Bacc("TRN2", target_bir_lowering=False, debug=False,
                   enable_asserts=False, num_devices=N_CORES)

    # ---------------- DRAM I/O ----------------
    x_d = nc.dram_tensor("x", [BL, DIM, H, W], gdt, kind="ExternalInput").ap()
    x32_d = nc.dram_tensor("x32", [BL, DIM, H, W], f32, kind="ExternalInput").ap()
    out_d = nc.dram_tensor("out", [BL, DIM, H, W], f32, kind="ExternalOutput").ap()
    wq_d = nc.dram_tensor("wq_t", [DIM, DIM], gdt, kind="ExternalInput").ap()
    wk_d = nc.dram_tensor("wk_t", [DIM, DIM], gdt, kind="ExternalInput").ap()
    wv_d = nc.dram_tensor("wv_t", [DIM, DIM], gdt, kind="ExternalInput").ap()
    wp_d = nc.dram_tensor("proj_t", [DIM, DIM], gdt, kind="ExternalInput").ap()
    # diagonal tap matrices for PE convs
    dg1_d = [nc.dram_tensor(n, [NH, 9, HD, HD], cdt, kind="ExternalInput").ap()
             for n in ("diag1q", "diag1k")]
    dg2_d = nc.dram_tensor("diag2", [NH, 9, HD, HD], cdt, kind="ExternalInput").ap()
    # tap weight vectors for DVE/GPS convs: [NH, HD, 9]
    wv1_d = [nc.dram_tensor(n, [NH, HD, 9], f32, kind="ExternalInput").ap()
             for n in ("wvec1q", "wvec1k")]
    # biases as [DIM, 1]
    b1_d = [nc.dram_tensor(n, [DIM, 1], f32, kind="ExternalInput").ap()
            for n in ("sq_b", "sk_b")]
    dwcb_d = nc.dram_tensor("dwc_b", [DIM, 1], f32, kind="ExternalInput").ap()
    projb_d = nc.dram_tensor("proj_b", [DIM, 1], f32, kind="ExternalInput").ap()
    # SE weights: w1 [NH, HD, HD4] (lhsT, prescaled by 1/npix), b1 [NH, HD4, 1]
    #             w2 [NH, HD4, HD] (lhsT), b2 [NH, HD, 1]
    sew1_d = [nc.dram_tensor(n, [NH, HD, HD4], f32, kind="ExternalInput").ap()
              for n in ("se_w1q", "se_w1k")]
    seb1_d = [nc.dram_tensor(n, [NH, HD4, 1], f32, kind="ExternalInput").ap()
              for n in ("se_b1q", "se_b1k")]
    sew2_d = [nc.dram_tensor(n, [NH, HD4, HD], f32, kind="ExternalInput").ap()
              for n in ("se_w2q", "se_w2k")]
    seb2_d = [nc.dram_tensor(n, [NH, HD, 1], f32, kind="ExternalInput").ap()
              for n in ("se_b2q", "se_b2k")]

    with tile.TileContext(nc) as tc, ExitStack() as ctx:
        const = ctx.enter_context(tc.tile_pool(name="const", bufs=1))
        big = ctx.enter_context(tc.tile_pool(name="big", bufs=1))
        wpool = ctx.enter_context(tc.tile_pool(name="wpool", bufs=1))
        xpool = ctx.enter_context(tc.tile_pool(name="xpool", bufs=2))
        vpool = ctx.enter_context(tc.tile_pool(name="vpool", bufs=2))
        o2pool = ctx.enter_context(tc.tile_pool(name="o2pool", bufs=2))
        otpool = ctx.enter_context(tc.tile_pool(name="otpool", bufs=2))
        dgpool = ctx.enter_context(tc.tile_pool(name="dgpool", bufs=1))
        accpool = ctx.enter_context(tc.tile_pool(name="accpool", bufs=1))
        statpool = ctx.enter_context(tc.tile_pool(name="statpool", bufs=2))
        mmpool = ctx.enter_context(tc.tile_pool(name="mmpool", bufs=5, space="PSUM"))
        sepool = ctx.enter_context(tc.tile_pool(name="sepool", bufs=2, space="PSUM"))

        # ---------- persistent SBUF ----------
        TPAD = TH * WP                     # conv psum width (full padded rows)
        qpad = [big.tile([HD, PADN + 2], cdt, name=f"qpad{c}") for c in range(NH)]
        dwq = [big.tile([HD, PADN + 2], cdt, name=f"dwq{c}") for c in range(NH)]
        dwk = [big.tile([HD, PADN + 2], cdt, name=f"dwk{c}") for c in range(NH)]
        qpad3 = [t[:, 1:1 + PADN].rearrange("p (h w) -> p h w", w=WP) for t in qpad]
        dwq3 = [t[:, 1:1 + PADN].rearrange("p (h w) -> p h w", w=WP) for t in dwq]
        dwk3 = [t[:, 1:1 + PADN].rearrange("p (h w) -> p h w", w=WP) for t in dwk]

        # zero whole buffers once (interiors are fully rewritten per use;
        # halo borders + slop cells must stay zero)
        for tt in qpad + dwq + dwk:
            nc.vector.memset(tt, 0.0)

        # small constants
        bias1 = [[const.tile([HD, 1], f32, name=f"b1_{br}_{c}") for c in range(NH)]
                 for br in range(2)]
        dwcb = [const.tile([HD, 1], f32, name=f"dwcb{c}") for c in range(NH)]
        projb = [const.tile([HD, 1], f32, name=f"projb{c}") for c in range(NH)]
        for c in range(NH):
            sl = slice(c * HD, (c + 1) * HD)
            for br in range(2):
                nc.sync.dma_start(bias1[br][c], b1_d[br][sl])
            nc.sync.dma_start(dwcb[c], dwcb_d[sl])
            nc.sync.dma_start(projb[c], projb_d[sl])
        sew1 = [[const.tile([HD, HD4], f32, name=f"sew1_{br}_{c}") for c in range(NH)]
                for br in range(2)]
        seb1 = [[const.tile([HD4, 1], f32, name=f"seb1_{br}_{c}") for c in range(NH)]
                for br in range(2)]
        sew2 = [[const.tile([HD4, HD], f32, name=f"sew2_{br}_{c}") for c in range(NH)]
                for br in range(2)]
        seb2 = [[const.tile([HD, 1], f32, name=f"seb2_{br}_{c}") for c in range(NH)]
                for br in range(2)]
        for br in range(2):
            for c in range(NH):
                nc.sync.dma_start(sew1[br][c], sew1_d[br][c])
                nc.sync.dma_start(seb1[br][c], seb1_d[br][c])
                nc.sync.dma_start(sew2[br][c], sew2_d[br][c])
                nc.sync.dma_start(seb2[br][c], seb2_d[br][c])

        def taps_views(p3, r0):
            """9 shifted input views for output rows r0..r0+TH-1 (padded buf)."""
            return [p3[:, r0 + dy + 1: r0 + dy + 1 + TH, dx + 1: dx + 1 + W]
                    for (dy, dx) in TAPS]

        def taps_flat(tbuf, r0):
            """9 contiguous input slices (full padded rows incl. slop) for
            padded-space conv over output padded rows r0+1..r0+TH."""
            base = 1 + (r0 + 1) * WP
            return [tbuf[:, base + dy * WP + dx: base + dy * WP + dx + TPAD]
                    for (dy, dx) in TAPS]

        def taps_views_full(p3):
            return [p3[:, dy + 1: dy + 1 + H, dx + 1: dx + 1 + W]
                    for (dy, dx) in TAPS]

        def emit_body(rep):
            sfx = f"_r{rep}" if cfg['repeat'] > 1 else ""
            s_scale = [[None] * NH for _ in range(2)]   # sigmoid outputs

            for b in range(BL):
                # ================= PHASE 1: q then k =================
                for br in range(2):
                    w_d = wq_d if br == 0 else wk_d
                    pad3 = qpad3
                    pad_f = qpad
                    dst3 = dwq3 if br == 0 else dwk3
                    w_sb = []
                    for kc in range(NH):
                        row = []
                        for oc in range(NH):
                            wt = wpool.tile([HD, HD], gdt, tag=f"wA{kc}_{oc}",
                                            name=f"wA{kc}_{oc}_b{b}_{br}{sfx}")
                            nc.sync.dma_start(wt, w_d[kc * HD:(kc + 1) * HD,
                                                      oc * HD:(oc + 1) * HD])
                            row.append(wt)
                        w_sb.append(row)
                    # ---- GEMM: branch channels over all spatial tiles ----
                    for t in range(NT):
                        r0 = t * TH
                        xt = []
                        for kc in range(NH):
                            xx = xpool.tile([HD, TN], gdt, tag=f"xt{kc}",
                                            name=f"xt{kc}_b{b}_{br}_{t}{sfx}")
                            nc.sync.dma_start(
                                xx.rearrange("p (h w) -> p h w", w=W),
                                x_d[b, kc * HD:(kc + 1) * HD, r0:r0 + TH, :])
                            xt.append(xx)
                        for oc in range(NH):
                            ps = mmpool.tile([HD, TN], f32, tag="mm",
                                             name=f"g{b}_{br}_{t}_{oc}{sfx}")
                            for kc in range(NH):
                                nc.tensor.matmul(
                                    ps, w_sb[kc][oc], xt[kc],
                                    start=(kc == 0), stop=(kc == NH - 1))
                            nc.scalar.copy(
                                pad3[oc][:, 1 + r0:1 + r0 + TH, 1:1 + W],
                                ps.rearrange("p (h w) -> p h w", w=W))
                    # ---- conv1 + fused pooling ----
                    for oc in range(NH):
                        eng = cfg['conv1_assign'][(br, oc)]
                        if eng == 'pe':
                            stats = statpool.tile([HD, NT], f32, tag="stats",
                                                  name=f"st{b}_{br}_{oc}{sfx}")
                            dgs = []
                            for j in range(9):
                                dg = dgpool.tile([HD, HD], cdt, tag=f"dg{j}",
                                                 name=f"dg{j}_{b}_{br}_{oc}{sfx}")
                                nc.sync.dma_start(dg, dg1_d[br][oc, j])
                                dgs.append(dg)
                            for t in range(NT):
                                r0 = t * TH
                                if cfg['conv_flat']:
                                    ps = mmpool.tile([HD, TPAD], f32, tag="mm",
                                                     name=f"c1{b}_{br}_{t}_{oc}{sfx}")
                                    for j, v in enumerate(taps_flat(pad_f[oc], r0)):
                                        nc.tensor.matmul(ps, dgs[j], v,
                                                         start=(j == 0), stop=(j == 8))
                                    psv = ps.rearrange("p (h w) -> p h w", w=WP)[:, :, 1:1 + W]
                                else:
                                    ps = mmpool.tile([HD, TN], f32, tag="mm",
                                                     name=f"c1{b}_{br}_{t}_{oc}{sfx}")
                                    for j, v in enumerate(taps_views(pad3[oc], r0)):
                                        nc.tensor.matmul(ps, dgs[j], v,
                                                         start=(j == 0), stop=(j == 8))
                                    psv = ps.rearrange("p (h w) -> p h w", w=W)
                                nc.scalar.activation(
                                    dst3[oc][:, 1 + r0:1 + r0 + TH, 1:1 + W],
                                    psv,
                                    AF.Identity, bias=bias1[br][oc],
                                    accum_out=stats[:, t:t + 1])
                            pooled_w = NT
                        else:
                            stats = statpool.tile([HD, NT], f32, tag="stats",
                                                  name=f"st{b}_{br}_{oc}{sfx}")
                            wvec = dgpool.tile([HD, 9], f32, tag="wvec",
                                               name=f"wv{b}_{br}_{oc}{sfx}")
                            nc.sync.dma_start(wvec, wv1_d[br][oc])
                            e = nc.vector if eng == 'dve' else nc.gpsimd
                            views = taps_views_full(pad3[oc])
                            acc = [accpool.tile([HD, NPIX], cdt, tag=f"acc{i}",
                                                name=f"acc{i}_{b}_{br}_{oc}{sfx}")
                                   for i in range(2)]
                            a3 = [a.rearrange("p (h w) -> p h w", w=W) for a in acc]
                            # tap 0 (+ bias), taps 1..7 ping-pong, tap 8 -> dst
                            e.tensor_scalar(a3[0], views[0], wvec[:, 0:1],
                                            bias1[br][oc], AL.mult, AL.add)
                            cur = 0
                            for j in range(1, 8):
                                e.scalar_tensor_tensor(
                                    a3[1 - cur], views[j], wvec[:, j:j + 1],
                                    a3[cur], AL.mult, AL.add)
                                cur = 1 - cur
                            e.scalar_tensor_tensor(
                                dst3[oc][:, 1:1 + H, 1:1 + W], views[8],
                                wvec[:, 8:9], a3[cur], AL.mult, AL.add,
                                accum_out=stats[:, 0:1])
                            pooled_w = 1
                        # ---- SE for this head ----
                        pooled = const.tile([HD, 1], f32, tag="pooled",
                                            bufs=4, name=f"pool{b}_{br}_{oc}{sfx}")
                        nc.vector.tensor_reduce(pooled, stats[:, 0:pooled_w],
                                                mybir.AxisListType.X, AL.add)
                        ps1 = sepool.tile([HD4, 1], f32, tag="se",
                                          name=f"se1_{b}_{br}_{oc}{sfx}")
                        nc.tensor.matmul(ps1, sew1[br][oc], pooled,
                                         start=True, stop=True)
                        hvec = const.tile([HD4, 1], f32, tag="hvec", bufs=4,
                                          name=f"h{b}_{br}_{oc}{sfx}")
                        nc.scalar.activation(hvec, ps1, AF.Relu,
                                             bias=seb1[br][oc])
                        ps2 = sepool.tile([HD, 1], f32, tag="se",
                                          name=f"se2_{b}_{br}_{oc}{sfx}")
                        nc.tensor.matmul(ps2, sew2[br][oc], hvec,
                                         start=True, stop=True)
                        s_sb = const.tile([HD, 1], f32, tag="s_scale", bufs=16,
                                          name=f"s{b}_{br}_{oc}{sfx}")
                        nc.scalar.activation(s_sb, ps2, AF.Sigmoid,
                                             bias=seb2[br][oc])
                        s_scale[br][oc] = s_sb

                # ============ PHASE 1.5: m = s_q*dwq + s_k*dwk (into dwk) ====
                for oc in range(NH):
                    scr = accpool.tile([HD, PADN + 2], cdt, tag="acc0",
                                       name=f"scr{b}_{oc}{sfx}")
                    nc.vector.tensor_scalar(scr, dwk[oc], s_scale[1][oc], None,
                                            AL.mult)
                    nc.vector.scalar_tensor_tensor(dwk[oc], dwq[oc],
                                                   s_scale[0][oc], scr,
                                                   AL.mult, AL.add)

                # ================= PHASE 2 =================
                wv_sb, wp_sb = [], []
                for kc in range(NH):
                    rowv, rowp = [], []
                    for oc in range(NH):
                        wt = wpool.tile([HD, HD], gdt, tag=f"wA{kc}_{oc}",
                                        name=f"wV{kc}_{oc}_b{b}{sfx}")
                        nc.sync.dma_start(wt, wv_d[kc * HD:(kc + 1) * HD,
                                                   oc * HD:(oc + 1) * HD])
                        rowv.append(wt)
                        wt2 = wpool.tile([HD, HD], gdt, tag=f"wB{kc}_{oc}",
                                         name=f"wP{kc}_{oc}_b{b}{sfx}")
                        nc.sync.dma_start(wt2, wp_d[kc * HD:(kc + 1) * HD,
                                                    oc * HD:(oc + 1) * HD])
                        rowp.append(wt2)
                    wv_sb.append(rowv)
                    wp_sb.append(rowp)
                dg2 = []
                for oc in range(NH):
                    row = []
                    for j in range(9):
                        dg = dgpool.tile([HD, HD], cdt, tag=f"dg2_{oc}_{j}",
                                         name=f"dg2_{oc}_{j}_b{b}{sfx}")
                        nc.sync.dma_start(dg, dg2_d[oc, j])
                        row.append(dg)
                    dg2.append(row)
                for t in range(NT):
                    r0 = t * TH
                    xt, xrt = [], []
                    for kc in range(NH):
                        xx = xpool.tile([HD, TN], gdt, tag=f"xt{kc}",
                                        name=f"x2_{kc}_b{b}_{t}{sfx}")
                        nc.sync.dma_start(
                            xx.rearrange("p (h w) -> p h w", w=W),
                            x_d[b, kc * HD:(kc + 1) * HD, r0:r0 + TH, :])
                        xt.append(xx)
                        xr = xpool.tile([HD, TN], f32, tag=f"x32_{kc}",
                                        name=f"xr_{kc}_b{b}_{t}{sfx}")
                        nc.sync.dma_start(
                            xr.rearrange("p (h w) -> p h w", w=W),
                            x32_d[b, kc * HD:(kc + 1) * HD, r0:r0 + TH, :])
                        xrt.append(xr)
                    v_sb = []
                    for oc in range(NH):
                        ps = mmpool.tile([HD, TN], f32, tag="mm",
                                         name=f"v{b}_{t}_{oc}{sfx}")
                        for kc in range(NH):
                            nc.tensor.matmul(
                                ps, wv_sb[kc][oc], xt[kc],
                                start=(kc == 0), stop=(kc == NH - 1))
                        vv = vpool.tile([HD, TN], gdt, tag=f"vt{oc}",
                                        name=f"vt{oc}_b{b}_{t}{sfx}")
                        nc.scalar.copy(vv, ps)
                        v_sb.append(vv)
                    o2 = []
                    for oc in range(NH):
                        if cfg['conv_flat']:
                            ps = mmpool.tile([HD, TPAD], f32, tag="mm",
                                             name=f"c2{b}_{t}_{oc}{sfx}")
                            for j, v in enumerate(taps_flat(dwk[oc], r0)):
                                nc.tensor.matmul(ps, dg2[oc][j], v,
                                                 start=(j == 0), stop=(j == 8))
                            psv = ps.rearrange("p (h w) -> p h w", w=WP)[:, :, 1:1 + W]
                        else:
                            ps = mmpool.tile([HD, TN], f32, tag="mm",
                                             name=f"c2{b}_{t}_{oc}{sfx}")
                            for j, v in enumerate(taps_views(dwk3[oc], r0)):
                                nc.tensor.matmul(ps, dg2[oc][j], v,
                                                 start=(j == 0), stop=(j == 8))
                            psv = ps.rearrange("p (h w) -> p h w", w=W)
                        c2t = o2pool.tile([HD, TN], gdt, tag=f"c2t_{oc}",
                                          name=f"c2t_{oc}_b{b}_{t}{sfx}")
                        nc.scalar.activation(
                            c2t.rearrange("p (h w) -> p h w", w=W), psv,
                            AF.Identity, bias=dwcb[oc])
                        oo = o2pool.tile([HD, TN], gdt, tag=f"o2_{oc}",
                                         name=f"o2_{oc}_b{b}_{t}{sfx}")
                        # (conv2 + dwc_b) * v
                        nc.vector.tensor_mul(oo, c2t, v_sb[oc])
                        o2.append(oo)
                    for oc in range(NH):
                        ps = mmpool.tile([HD, TN], f32, tag="mm",
                                         name=f"p{b}_{t}_{oc}{sfx}")
                        for kc in range(NH):
                            nc.tensor.matmul(
                                ps, wp_sb[kc][oc], o2[kc],
                                start=(kc == 0), stop=(kc == NH - 1))
                        ot = otpool.tile([HD, TN], f32, tag=f"ot{oc}",
                                         name=f"ot{oc}_b{b}_{t}{sfx}")
                        # (proj + proj_b) + x  (residual)
                        nc.vector.scalar_tensor_tensor(ot, ps, projb[oc],
                                                       xrt[oc], AL.add, AL.add)
                        nc.sync.dma_start(
                            out_d[b, oc * HD:(oc + 1) * HD, r0:r0 + TH, :],
                            ot.rearrange("p (h w) -> p h w", w=W))

        if cfg['repeat'] > 1:
            for rep in range(cfg['repeat']):
                emit_body(rep)
        else:
            emit_body(0)

    nc.compile()
    return nc


# ---------------------------------------------------------------------------
# host-side weight prep
# ---------------------------------------------------------------------------

def prep_weights(inputs, cfg):
    cdt = np.dtype('bfloat16') if False else None  # placeholder
    import ml_dtypes
    conv_np = ml_dtypes.bfloat16 if cfg['conv_bf16'] else np.float32
    f32 = np.float32
    bf = ml_dtypes.bfloat16 if cfg.get('gemm_bf16', True) else np.float32
    qkv_w = np.asarray(inputs['qkv_w'], f32)
    wq_t = np.ascontiguousarray(qkv_w[0:DIM].T).astype(bf)
    wk_t = np.ascontiguousarray(qkv_w[DIM:2 * DIM].T).astype(bf)
    wv_t = np.ascontiguousarray(qkv_w[2 * DIM:3 * DIM].T).astype(bf)
    proj_t = np.ascontiguousarray(np.asarray(inputs['proj_w'], f32).T).astype(bf)

    def diag_taps(wconv):
        w = np.asarray(wconv, f32).reshape(DIM, 9)    # [(dy,dx) row-major]
        out = np.zeros((NH, 9, HD, HD), f32)
        idx = np.arange(HD)
        for c in range(NH):
            for j in range(9):
                out[c, j, idx, idx] = w[c * HD:(c + 1) * HD, j]
        return out.astype(conv_np)

    def wvecs(wconv):
        w = np.asarray(wconv, f32).reshape(DIM, 9)
        return np.ascontiguousarray(w.reshape(NH, HD, 9))

    npix = cfg['H'] * W
    d = dict(
        wq_t=wq_t, wk_t=wk_t, wv_t=wv_t, proj_t=proj_t,
        diag1q=diag_taps(inputs['sq_w']),
        diag1k=diag_taps(inputs['sk_w']),
        diag2=diag_taps(inputs['dwc_w']),
        wvec1q=wvecs(inputs['sq_w']),
        wvec1k=wvecs(inputs['sk_w']),
        sq_b=np.asarray(inputs['sq_b'], f32).reshape(DIM, 1),
        sk_b=np.asarray(inputs['sk_b'], f32).reshape(DIM, 1),
        dwc_b=np.asarray(inputs['dwc_b'], f32).reshape(DIM, 1),
        proj_b=np.asarray(inputs['proj_b'], f32).reshape(DIM, 1),
        se_w1q=np.ascontiguousarray(
            np.asarray(inputs['cq_w1'], f32).transpose(0, 2, 1) / npix),
        se_b1q=np.asarray(inputs['cq_b1'], f32).reshape(NH, HD4, 1),
        se_w2q=np.ascontiguousarray(
            np.asarray(inputs['cq_w2'], f32).transpose(0, 2, 1)),
        se_b2q=np.asarray(inputs['cq_b2'], f32).reshape(NH, HD, 1),
        se_w1k=np.ascontiguousarray(
            np.asarray(inputs['ck_w1'], f32).transpose(0, 2, 1) / npix),
        se_b1k=np.asarray(inputs['ck_b1'], f32).reshape(NH, HD4, 1),
        se_w2k=np.ascontiguousarray(
            np.asarray(inputs['ck_w2'], f32).transpose(0, 2, 1)),
        se_b2k=np.asarray(inputs['ck_b2'], f32).reshape(NH, HD, 1),
    )
    return d


_CACHE = {}


def _get_compiled(cfg_key, cfg):
    if cfg_key not in _CACHE:
        _CACHE[cfg_key] = build_nc(cfg)
    return _CACHE[cfg_key]


def kernel(**inputs):
    import ml_dtypes
    from concourse import bass_utils
    cfg = default_cfg()
    nc = _get_compiled('main', cfg)
    w = prep_weights(inputs, cfg)
    x32 = np.asarray(inputs['x'], np.float32)
    x = x32.astype(ml_dtypes.bfloat16) if cfg['gemm_bf16'] else x32
    BL = cfg['b_local']
    in_maps = []
    for core in range(N_CORES):
        m = dict(w)
        m['x'] = np.ascontiguousarray(x[core * BL:(core + 1) * BL])
        m['x32'] = np.ascontiguousarray(x32[core * BL:(core + 1) * BL])
        in_maps.append(m)
    res = bass_utils.run_bass_kernel_spmd(nc, in_maps, core_ids=list(range(N_CORES)))
    out = np.empty((B, DIM, H_FULL, W), np.float32)
    for core in range(N_CORES):
        out[core * BL:(core + 1) * BL] = res.results[core]['out']
    return out
